# revision 11
# baseline (speedup 1.0000x reference)
"""LocalGLMnet forward kernel for Trainium2, 8-core data parallel.

Math (per batch row b):
  interim[i,j] = sigmoid( sum_{di,dj} x_pad[b, i+di, j+dj] * w[i,j,di,dj] )   (10,100)
  forecast[j]  = sum_i x[b,i,j] * interim[i,j]
  penalty[j]   = sum_i ETA * interim[i,j]^2           (ALPHA = 0 -> sqrt term vanishes)
  out[b] = [forecast, penalty]                         (2, 100)

Device mapping (per core, batch shard 2048 = 16 tiles of 128 rows):
  - batch on SBUF partitions.
  - conv as PE matmuls: for each x row r, stationary = x^T row slice
    (100 cols x 128 batch), moving = host-packed banded weight columns for the
    <=5 output rows i with |i-r| <= 2; accumulate in PSUM over r.
  - sigmoid (ScalarE, PSUM->SBUF), penalty pre-term Square(0.1*s) = 0.01*s^2
    (ScalarE), decoded = x * s (VectorE), two grouped reductions over i
    (VectorE tensor_reduce axis=X on a strided view).
"""

import os
import numpy as np
import ml_dtypes

import concourse.bass as bass
import concourse.bacc as bacc
import concourse.tile as tile
from concourse import mybir
from concourse.bass_utils import run_bass_kernel_spmd
from concourse._compat import with_exitstack

N_CORES = 8
B = 16384
BPC = B // N_CORES          # 2048
LB, NA = 10, 100            # look_back (rows), n_ages (cols)
NTILE = BPC // 128          # 16
F = LB * NA                 # 1000
ETA = 0.01

F32 = mybir.dt.float32
BF16 = mybir.dt.bfloat16
BF16_NP = ml_dtypes.bfloat16

LAST_RESULTS = None         # BassKernelResults of the most recent run


def _schedule():
    """Matmul schedule: one entry per (x-row r, psum bank).

    Returns (entries, total_cols). Entry: dict with
      r: x row used as stationary
      i0, i1: output-row range covered (inclusive)
      poff: psum column offset (bank0 = i in 0..4 at i*100, bank1 = i in 5..9
            at 512 + (i-5)*100)
      n: moving columns
      woff: column offset into the packed weight matrix
    """
    entries = []
    off = 0
    for r in range(LB):
        for bank, (lo, hi) in ((0, (0, 4)), (1, (5, 9))):
            ivals = [i for i in range(lo, hi + 1) if r - 2 <= i <= r + 2]
            if not ivals:
                continue
            i0, i1 = ivals[0], ivals[-1]
            n = (i1 - i0 + 1) * NA
            poff = bank * 512 + (i0 - lo) * NA
            entries.append(dict(r=r, i0=i0, i1=i1, bank=bank, poff=poff,
                                n=n, woff=off))
            off += n
    return entries, off


SCHED, WTOT = _schedule()   # WTOT == 4400


def _pack_wm(weight):
    """Pack (10,100,5,5) unshared conv weight into the (100, WTOT) moving
    operand. Column (r; i, j) holds, at partition jp, the coefficient of
    x[b, r, jp] in interim[b, i, j]:  w[i, j, r-i+2, jp-j+2] (0 outside the
    5-tap window)."""
    w = np.asarray(weight, np.float32)
    wm = np.zeros((NA, WTOT), np.float32)
    for e in SCHED:
        r = e["r"]
        for k, i in enumerate(range(e["i0"], e["i1"] + 1)):
            di = r - i + 2
            blk = np.zeros((NA, NA), np.float32)   # [jp, j]
            for dj in range(5):
                # jp = j + dj - 2
                j_lo = max(0, 2 - dj)
                j_hi = min(NA, NA + 2 - dj)
                js = np.arange(j_lo, j_hi)
                blk[js + dj - 2, js] = w[i, js, di, dj]
            c0 = e["woff"] + k * NA
            wm[:, c0:c0 + NA] = blk
    return wm.astype(BF16_NP)


@with_exitstack
def _kernel_body(ctx, tc, o_ap, xn_ap, xt_ap, wm_ap, reps=1):
    nc = tc.nc
    wpool = ctx.enter_context(tc.tile_pool(name="wpool", bufs=1))
    pool = ctx.enter_context(tc.tile_pool(name="work", bufs=WORK_BUFS))
    pspool = ctx.enter_context(tc.tile_pool(name="ps", bufs=PSUM_BUFS,
                                            space="PSUM"))

    wm_sb = wpool.tile([NA, WTOT], BF16)
    nc.sync.dma_start(out=wm_sb[:], in_=wm_ap[:])

    last_for_bank = {}
    for k, e in enumerate(SCHED):
        last_for_bank[e["bank"]] = k

    if reps == 1:
        _one_pass(tc, pool, pspool, wm_sb, o_ap, xn_ap, xt_ap, last_for_bank,
                  wpool)
    else:
        # benchmarking only: repeat the whole pass on-device so device time
        # dominates host/RPC noise in wall-clock measurements
        with tc.For_i(0, reps, 1):
            _one_pass(tc, pool, pspool, wm_sb, o_ap, xn_ap, xt_ap,
                      last_for_bank, wpool)


VARIANT = dict(reduces="tree", matmuls=True, mult=True, xn_dma=True,
               xt_dma=True, act=True, xt_big=True)
WORK_BUFS = 4
PSUM_BUFS = 4


def _one_pass(tc, pool, pspool, wm_sb, o_ap, xn_ap, xt_ap, last_for_bank,
              wpool):
    nc = tc.nc
    V = VARIANT
    xt_all = None
    if V["xt_big"]:
        # one contiguous 4 MB load per pass instead of 16 strided tile loads
        xt_all = wpool.tile([NA, LB, BPC], BF16)
        nc.sync.dma_start(out=xt_all[:], in_=xt_ap[:])
    for t in range(NTILE):
        b0 = t * 128
        xn_t = pool.tile([128, F], F32)
        if V["xn_dma"]:
            nc.sync.dma_start(out=xn_t[:], in_=xn_ap[b0:b0 + 128, :])
        if V["xt_big"]:
            xt_t = xt_all[:, :, b0:b0 + 128]
        else:
            xt_t = pool.tile([NA, LB, 128], BF16)
            if V["xt_dma"]:
                nc.sync.dma_start(out=xt_t[:], in_=xt_ap[:, :, b0:b0 + 128])

        ps = pspool.tile([128, 1024], F32)
        if V["matmuls"]:
            started = {0: False, 1: False}
            for k, e in enumerate(SCHED):
                nc.tensor.matmul(
                    ps[:, e["poff"]:e["poff"] + e["n"]],
                    xt_t[:, e["r"], :],
                    wm_sb[:, e["woff"]:e["woff"] + e["n"]],
                    start=not started[e["bank"]],
                    stop=(k == last_for_bank[e["bank"]]),
                )
                started[e["bank"]] = True

        sig = pool.tile([128, F], F32)
        sq = pool.tile([128, F], F32)
        if V["act"]:
            ps_v = ps[:].rearrange("p (h f) -> p h f", h=2)[:, :, 0:500]
            sig_v = sig[:].rearrange("p (h f) -> p h f", h=2)
            nc.scalar.activation(sig_v, ps_v,
                                 mybir.ActivationFunctionType.Sigmoid)
            # penalty pre-term: Square(0.1 * s) = 0.01 * s^2 = ETA * s^2
            nc.scalar.activation(sq[:], sig[:],
                                 mybir.ActivationFunctionType.Square, scale=0.1)

        d = pool.tile([128, F], F32)
        if V["mult"]:
            nc.vector.tensor_tensor(out=d[:], in0=xn_t[:], in1=sig[:],
                                    op=mybir.AluOpType.mult)

        fp = pool.tile([128, 2, NA], F32)
        red = V["reduces"]
        if red == "strided":
            d_v = d[:].rearrange("p (i j) -> p j i", i=LB)
            nc.vector.tensor_reduce(out=fp[:, 0, :], in_=d_v,
                                    axis=mybir.AxisListType.X,
                                    op=mybir.AluOpType.add)
            sq_v = sq[:].rearrange("p (i j) -> p j i", i=LB)
            nc.vector.tensor_reduce(out=fp[:, 1, :], in_=sq_v,
                                    axis=mybir.AxisListType.X,
                                    op=mybir.AluOpType.add)
        elif red == "contig":
            # timing probe only (reduces over the wrong axis, same elem count)
            d_v = d[:].rearrange("p (i j) -> p i j", i=LB)
            nc.vector.tensor_reduce(out=fp[:, 0, 0:LB], in_=d_v,
                                    axis=mybir.AxisListType.X,
                                    op=mybir.AluOpType.add)
            sq_v = sq[:].rearrange("p (i j) -> p i j", i=LB)
            nc.vector.tensor_reduce(out=fp[:, 1, 0:LB], in_=sq_v,
                                    axis=mybir.AxisListType.X,
                                    op=mybir.AluOpType.add)
        elif red == "tree":
            # contiguous pairwise tree: forecast on DVE, penalty on GpSimd
            ta = pool.tile([128, 5, NA], F32)
            nc.vector.tensor_tensor(out=ta[:], in0=d[:, 0:500], in1=d[:, 500:1000],
                                    op=mybir.AluOpType.add)
            tb = pool.tile([128, 2, NA], F32)
            nc.vector.tensor_tensor(out=tb[:], in0=ta[:, 0:2, :], in1=ta[:, 2:4, :],
                                    op=mybir.AluOpType.add)
            tcq = pool.tile([128, NA], F32)
            nc.vector.tensor_tensor(out=tcq[:], in0=tb[:, 0, :], in1=tb[:, 1, :],
                                    op=mybir.AluOpType.add)
            nc.vector.tensor_tensor(out=fp[:, 0, :], in0=tcq[:], in1=ta[:, 4, :],
                                    op=mybir.AluOpType.add)
            ga = pool.tile([128, 5, NA], F32)
            nc.gpsimd.tensor_tensor(out=ga[:], in0=sq[:, 0:500], in1=sq[:, 500:1000],
                                    op=mybir.AluOpType.add)
            gb = pool.tile([128, 2, NA], F32)
            nc.gpsimd.tensor_tensor(out=gb[:], in0=ga[:, 0:2, :], in1=ga[:, 2:4, :],
                                    op=mybir.AluOpType.add)
            gq = pool.tile([128, NA], F32)
            nc.gpsimd.tensor_tensor(out=gq[:], in0=gb[:, 0, :], in1=gb[:, 1, :],
                                    op=mybir.AluOpType.add)
            nc.gpsimd.tensor_tensor(out=fp[:, 1, :], in0=gq[:], in1=ga[:, 4, :],
                                    op=mybir.AluOpType.add)
        elif red == "none":
            pass

        nc.sync.dma_start(out=o_ap[b0:b0 + 128, :, :], in_=fp[:])


_COMPILED = {}


def _get_compiled(reps=1):
    key = (reps, tuple(sorted(VARIANT.items())))
    if key not in _COMPILED:
        nc = bacc.Bacc("TRN2", target_bir_lowering=False, debug=False)
        xn = nc.dram_tensor("xn", [BPC, F], F32, kind="ExternalInput").ap()
        xt = nc.dram_tensor("xt", [NA, LB, BPC], BF16, kind="ExternalInput").ap()
        wm = nc.dram_tensor("wm", [NA, WTOT], BF16, kind="ExternalInput").ap()
        o = nc.dram_tensor("o", [BPC, 2, NA], F32, kind="ExternalOutput").ap()
        with tile.TileContext(nc) as tc:
            _kernel_body(tc, o, xn, xt, wm, reps=reps)
        nc.compile()
        _COMPILED[key] = nc
    return _COMPILED[key]


def kernel(x, weight):
    global LAST_RESULTS
    x = np.asarray(x, np.float32)
    weight = np.asarray(weight, np.float32)
    assert x.shape == (B, LB, NA), x.shape

    nc = _get_compiled()
    wm = _pack_wm(weight)

    in_maps = []
    for c in range(N_CORES):
        sh = x[c * BPC:(c + 1) * BPC]
        xn = np.ascontiguousarray(sh.reshape(BPC, F))
        xt = np.ascontiguousarray(sh.transpose(2, 1, 0)).astype(BF16_NP)
        in_maps.append({"xn": xn, "xt": xt, "wm": wm})

    trace = bool(int(os.environ.get("K_TRACE", "0")))
    res = run_bass_kernel_spmd(nc, in_maps, list(range(N_CORES)), trace=trace)
    LAST_RESULTS = res
    out = np.concatenate([res.results[c]["o"] for c in range(N_CORES)], axis=0)
    return out
